# revision 20
# baseline (speedup 1.0000x reference)
"""AGDN (2-layer GAT-style message passing) distributed Bass kernel for 8 TRN2 cores.

Strategy: shard dst nodes (and their incoming edges) across the 8 cores.
Edges are dst-sorted and packed host-side into "slots" of R=4 edges sharing a
dst AND a src-quarter; slots are packed into tiles of <=128 output nodes x 4
groups x 128 slots (group g = src-quarter g).  Per tile the device:
  - dma_gathers (ucode path, int16 quarter-local indices) table rows
    [x_lin | ones | al | ar] for edge sources, one gather per quarter,
  - computes w = exp(leaky_relu(al_src + ar_dst)) per edge,
  - aggregates per-node sums (numerator and softmax denominator together) with
    one-hot matmuls on the TensorEngine accumulating in PSUM,
  - normalizes, adds residual, applies elu (layer 1), scatters to DRAM.
Between layers: node features are re-projected (Phase A2) and AllGathered so
every core has the full gather table.
"""

import numpy as np

import concourse.bass as bass
import concourse.bacc as bacc
import concourse.mybir as mybir
import concourse.tile as tile
from concourse import bass_utils

F32 = mybir.dt.float32
I32 = mybir.dt.int32
I16 = mybir.dt.int16
AF = mybir.ActivationFunctionType
OP = mybir.AluOpType
AX = mybir.AxisListType

# problem constants
IN, HID, HEADS, OUT = 128, 16, 4, 64
SLOPE = 0.2
NC = 8
NQ = 4                  # src quarters (2 cores each)
R = 4                   # edges per slot (same dst, same src-quarter)
GROUPS = 4              # slot groups per tile; group g = src-quarter g
BLOCKS = GROUPS * R     # 16 gather blocks per tile
ROW = 128               # table row f32 elems (512B): x(64)|ones(H)|al(H)|ar(H)|pad
A16 = 176               # int16 aux cols: 4*32 src | 32 ar | 8 xr1 | 8 xr2
A32 = 26                # int32 aux cols (20:24 ld, 24 out_l, 25 out_g)

_CACHE = {}
DEBUG = False


def _set_dims(n, e):
    global N, E, NLOC, NPAD, HBLK, NBLK, QROWS
    N, E = n, e
    NLOC = N // NC
    NPAD = NLOC + 4
    HBLK = ((NPAD + 127) // 128) * 128
    NBLK = HBLK // 128
    QROWS = 2 * NPAD
    assert QROWS < 32768


_set_dims(100000, 1600000)


def _wrap16(idx):
    """[n] -> int16 [128, n//16] replicated-8 layout for dma_gather."""
    n = idx.shape[0]
    blk = idx.reshape(n // 16, 16).T.astype(np.int16)
    return np.tile(blk, (8, 1))


# --------------------------------------------------------------------------
# host-side graph packing
# --------------------------------------------------------------------------

def _pack_core(src_g, dst_l, core):
    # sort by (dst, src-quarter)
    quarter = src_g // (2 * NLOC)
    key = dst_l * NQ + quarter
    korder = np.argsort(key, kind="stable")
    src_g = src_g[korder]
    key = key[korder]
    cnt = np.bincount(key, minlength=NLOC * NQ)          # [node*NQ + q]
    estart = np.concatenate([[0], np.cumsum(cnt)])[:-1]
    nslot_nq = ((cnt + R - 1) // R).reshape(NLOC, NQ)    # [node, q]

    # tile boundaries: greedy over nodes; per-quarter slot capacity 128
    cum_q = np.cumsum(nslot_nq, axis=0)
    tiles = []
    n0 = 0
    base = np.zeros(NQ, np.int64)
    while n0 < NLOC:
        n_hi = min(n0 + 128, NLOC)
        ok = ((cum_q[n0:n_hi] - base[None, :]) <= 128).all(axis=1)
        k = int(np.argmin(ok)) if not ok.all() else n_hi - n0
        n1 = n0 + max(k, 1)
        if (nslot_nq[n0] > 128).any():
            raise ValueError("node with too many edges in one quarter")
        tiles.append((n0, n1))
        base = cum_q[n1 - 1].copy()
        n0 = n1
    T = len(tiles)

    aux16 = np.zeros((T, 128, A16), np.int16)
    aux32 = np.zeros((T, 128, A32), np.int32)
    zq = NLOC   # quarter-local / local zero row
    for t, (nlo, nhi) in enumerate(tiles):
        ld = np.full((128, GROUPS), 255.0, np.float32)
        srcidx = np.full((GROUPS, R, 128), zq, np.int64)   # [g, r, p]
        aridx = np.full((GROUPS, 128), zq, np.int64)       # [g, p]
        for g in range(NQ):
            p = 0
            for d in range(nlo, nhi):
                ns = nslot_nq[d, g]
                if ns == 0:
                    continue
                e0 = estart[d * NQ + g]
                ne = cnt[d * NQ + g]
                for s in range(ns):
                    tk = min(R, ne - R * s)
                    rows = src_g[e0 + R * s : e0 + R * s + tk]
                    srcidx[g, :tk, p] = (rows // NLOC % 2) * NPAD + rows % NLOC
                    ld[p, g] = d - nlo
                    aridx[g, p] = d
                    p += 1
            assert p <= 128
        for g in range(GROUPS):
            aux16[t, :, 32 * g : 32 * (g + 1)] = _wrap16(srcidx[g].reshape(-1))
        aux16[t, :, 128:160] = _wrap16(aridx.reshape(-1))
        out_l = np.arange(128) + nlo
        pad = out_l >= nhi
        out_l[pad] = NLOC + (np.arange(128)[pad] % 4)
        aux16[t, :, 160:168] = _wrap16(out_l)
        aux16[t, :, 168:176] = _wrap16(out_l)
        aux32[t, :, 20:24] = ld.view(np.int32)
        aux32[t, :, 24] = out_l
    return aux16, aux32, T


def _prepare(edge_index):
    src = edge_index[0].astype(np.int64)
    dst = edge_index[1].astype(np.int64)
    core = dst // NLOC
    a16s, a32s = [], []
    for c in range(NC):
        m = core == c
        a16, a32, _ = _pack_core(src[m], dst[m] - c * NLOC, c)
        a16s.append(a16)
        a32s.append(a32)
    Tmax = max(a.shape[0] for a in a16s)
    o16 = np.zeros((NC, Tmax, 128, A16), np.int16)
    o32 = np.zeros((NC, Tmax, 128, A32), np.int32)
    for c in range(NC):
        t_ = a16s[c].shape[0]
        o16[c, :t_] = a16s[c]
        o32[c, :t_] = a32s[c]
        if t_ < Tmax:
            o16[c, t_:] = np.int16(NLOC)       # all gathers hit zero rows
            o32[c, t_:, :, 20:24] = np.float32(255.0).view(np.int32)
            o32[c, t_:, :, 24] = NLOC
    return o16, o32, Tmax


# --------------------------------------------------------------------------
# device kernel
# --------------------------------------------------------------------------

def _build(T):
    nc = bacc.Bacc("TRN2", target_bir_lowering=False, num_devices=NC,
                   num_swdge_queues=4)

    xT = nc.declare_dram_parameter("xT", [128, HBLK], F32, isOutput=False)
    aux16_d = nc.declare_dram_parameter("aux16", [T, 128, A16], I16, isOutput=False)
    aux32_d = nc.declare_dram_parameter("aux32", [T, 128, A32], I32, isOutput=False)
    w1 = nc.declare_dram_parameter("w1", [IN, OUT], F32, isOutput=False)
    rw1 = nc.declare_dram_parameter("rw1", [IN, OUT], F32, isOutput=False)
    atl1 = nc.declare_dram_parameter("atl1", [128, OUT], F32, isOutput=False)
    atr1 = nc.declare_dram_parameter("atr1", [128, OUT], F32, isOutput=False)
    b1 = nc.declare_dram_parameter("b1", [128, OUT], F32, isOutput=False)
    w2 = nc.declare_dram_parameter("w2", [OUT, OUT], F32, isOutput=False)
    atl2 = nc.declare_dram_parameter("atl2", [128, OUT], F32, isOutput=False)
    atr2 = nc.declare_dram_parameter("atr2", [128, OUT], F32, isOutput=False)
    b2 = nc.declare_dram_parameter("b2", [128, OUT], F32, isOutput=False)
    iota_d = nc.declare_dram_parameter("iota", [128, 128], F32, isOutput=False)
    ident_d = nc.declare_dram_parameter("ident", [128, 128], F32, isOutput=False)
    out_d = nc.declare_dram_parameter("out", [NPAD, OUT], F32, isOutput=True)
    dbg = {}
    if DEBUG:
        dbg["t1"] = nc.declare_dram_parameter("d_t1", [NC * NPAD, ROW], F32, isOutput=True)
        dbg["h"] = nc.declare_dram_parameter("d_h", [HBLK, OUT], F32, isOutput=True)
        dbg["X0"] = nc.declare_dram_parameter("d_X0", [128, R * ROW], F32, isOutput=True)
        dbg["AR0"] = nc.declare_dram_parameter("d_AR0", [128, GROUPS * ROW], F32, isOutput=True)
        dbg["W0"] = nc.declare_dram_parameter("d_W0", [128, BLOCKS * HEADS], F32, isOutput=True)
        dbg["P0"] = nc.declare_dram_parameter("d_P0", [128, OUT + HEADS], F32, isOutput=True)

    with tile.TileContext(nc) as tc:
        with (
            tc.tile_pool(name="dram", bufs=1, space="DRAM") as dram,
            tc.tile_pool(name="const", bufs=1) as cpool,
            tc.tile_pool(name="sba", bufs=3) as sba,
            tc.tile_pool(name="psum", bufs=2, space="PSUM") as pp,
            tc.tile_pool(name="sbb", bufs=4) as sbb,
        ):
            shard1 = dram.tile([NPAD, ROW], F32)
            t1 = dram.tile([NC * NPAD, ROW], F32)
            xres_l = dram.tile([NPAD, OUT], F32)
            h_l = dram.tile([HBLK, OUT], F32)
            shard2 = dram.tile([NPAD, ROW], F32)
            t2 = dram.tile([NC * NPAD, ROW], F32)

            def const_from(handle, shape, tag):
                t_ = cpool.tile(shape, F32, tag=tag, name=tag)
                nc.sync.dma_start(t_[:], handle[:])
                return t_

            w1_sb = const_from(w1, [IN, OUT], "c_w1")
            rw1_sb = const_from(rw1, [IN, OUT], "c_rw1")
            w2_sb = const_from(w2, [OUT, OUT], "c_w2")
            atl1_sb = const_from(atl1, [128, OUT], "c_atl1")
            atr1_sb = const_from(atr1, [128, OUT], "c_atr1")
            b1_sb = const_from(b1, [128, OUT], "c_b1")
            atl2_sb = const_from(atl2, [128, OUT], "c_atl2")
            atr2_sb = const_from(atr2, [128, OUT], "c_atr2")
            b2_sb = const_from(b2, [128, OUT], "c_b2")
            iota_sb = const_from(iota_d, [128, 128], "c_iota")
            ident_sb = const_from(ident_d, [128, 128], "c_ident")
            ones4_sb = cpool.tile([128, HEADS], F32)
            nc.vector.memset(ones4_sb[:], 1.0)
            zrow_sb = cpool.tile([4, ROW], F32)
            nc.vector.memset(zrow_sb[:], 0.0)

            # ---- zero h_l (scatter-add base) ----
            zblk_sb = cpool.tile([128, OUT], F32)
            nc.vector.memset(zblk_sb[:], 0.0)
            for k in range(NBLK):
                nc.sync.dma_start(h_l[128 * k : 128 * (k + 1), :], zblk_sb[:])

            # ---- phase A ----
            for k in range(NBLK):
                rows = min(128, NLOC - 128 * k)
                if rows <= 0:
                    break
                xk = sba.tile([128, 128], F32, tag="xk")
                nc.sync.dma_start(xk[:], xT[:, 128 * k : 128 * (k + 1)])
                p0 = pp.tile([128, OUT], F32, tag="pa")
                nc.tensor.matmul(p0[:], lhsT=xk[:], rhs=w1_sb[:], start=True, stop=True)
                p1 = pp.tile([128, OUT], F32, tag="pa")
                nc.tensor.matmul(p1[:], lhsT=xk[:], rhs=rw1_sb[:], start=True, stop=True)
                sh = sba.tile([128, ROW], F32, tag="sh")
                nc.vector.tensor_copy(sh[:, 0:OUT], p0[:])
                nc.vector.tensor_copy(sh[:, OUT : OUT + HEADS], ones4_sb[:])
                tal = sba.tile([128, OUT], F32, tag="tal")
                nc.vector.tensor_tensor(tal[:], p0[:], atl1_sb[:], OP.mult)
                nc.vector.tensor_reduce(
                    sh[:, OUT + HEADS : OUT + 2 * HEADS],
                    tal[:].rearrange("p (c h) -> p h c", h=HEADS),
                    AX.X, OP.add,
                )
                tar = sba.tile([128, OUT], F32, tag="tar")
                nc.vector.tensor_tensor(tar[:], p0[:], atr1_sb[:], OP.mult)
                nc.vector.tensor_reduce(
                    sh[:, OUT + 2 * HEADS : OUT + 3 * HEADS],
                    tar[:].rearrange("p (c h) -> p h c", h=HEADS),
                    AX.X, OP.add,
                )
                xr = sba.tile([128, OUT], F32, tag="xr")
                nc.vector.tensor_tensor(xr[:], p1[:], b1_sb[:], OP.add)
                lo = 128 * k
                nc.sync.dma_start(shard1[lo : lo + rows, 0 : OUT + 3 * HEADS],
                                  sh[:rows, 0 : OUT + 3 * HEADS])
                nc.sync.dma_start(xres_l[lo : lo + rows, :], xr[:rows, :])
            nc.sync.dma_start(shard1[NLOC:NPAD, :], zrow_sb[:, 0:ROW])

            nc.gpsimd.collective_compute(
                "AllGather", OP.bypass,
                replica_groups=[list(range(NC))],
                ins=[shard1.opt()], outs=[t1.opt()],
            )

            _edge_layer(nc, sbb, pp, aux16_d, aux32_d, t1, shard1, xres_l, h_l,
                        None, iota_sb, T, 1, dbg=dbg)
            if DEBUG:
                nc.sync.dma_start(dbg["t1"][:], t1[:])
                nc.sync.dma_start(dbg["h"][:], h_l[:])

            # ---- phase A2 ----
            for k in range(NBLK):
                rows = min(128, NLOC - 128 * k)
                hk = sba.tile([128, OUT], F32, tag="hk")
                nc.sync.dma_start(hk[:], h_l[128 * k : 128 * (k + 1), :])
                pt = pp.tile([OUT, 128], F32, tag="pt")
                nc.tensor.transpose(pt[:], hk[:], ident_sb[:])
                hT = sba.tile([OUT, 128], F32, tag="hT")
                nc.vector.tensor_copy(hT[:], pt[:])
                p2 = pp.tile([128, OUT], F32, tag="pa")
                nc.tensor.matmul(p2[:], lhsT=hT[:], rhs=w2_sb[:], start=True, stop=True)
                if rows <= 0:
                    continue
                sh2 = sba.tile([128, ROW], F32, tag="sh2")
                nc.vector.tensor_copy(sh2[:, 0:OUT], p2[:])
                nc.vector.tensor_copy(sh2[:, OUT : OUT + 1], ones4_sb[:, 0:1])
                t2l = sba.tile([128, OUT], F32, tag="t2l")
                nc.vector.tensor_tensor(t2l[:], p2[:], atl2_sb[:], OP.mult)
                nc.vector.tensor_reduce(sh2[:, OUT + 1 : OUT + 2], t2l[:], AX.X, OP.add)
                t2r = sba.tile([128, OUT], F32, tag="t2r")
                nc.vector.tensor_tensor(t2r[:], p2[:], atr2_sb[:], OP.mult)
                nc.vector.tensor_reduce(sh2[:, OUT + 2 : OUT + 3], t2r[:], AX.X, OP.add)
                lo = 128 * k
                nc.sync.dma_start(shard2[lo : lo + rows, 0 : OUT + 3],
                                  sh2[:rows, 0 : OUT + 3])
            nc.sync.dma_start(shard2[NLOC:NPAD, :], zrow_sb[:, 0:ROW])

            nc.gpsimd.collective_compute(
                "AllGather", OP.bypass,
                replica_groups=[list(range(NC))],
                ins=[shard2.opt()], outs=[t2.opt()],
            )

            _edge_layer(nc, sbb, pp, aux16_d, aux32_d, t2, shard2, shard2, out_d,
                        b2_sb, iota_sb, T, 2, dbg=dbg)

    nc.finalize()
    return nc


def _edge_layer(nc, sbb, pp, aux16_d, aux32_d, table, loc_tab, xr_tab, out_tab,
                bias_sb, iota_sb, T, layer, dbg=None):
    H = HEADS if layer == 1 else 1
    MW = OUT + H
    LW = BLOCKS * H
    CL, CR = OUT + H, OUT + 2 * H   # al, ar col offsets
    for t in range(T):
        tg = f"l{layer}"
        aux16t = sbb.tile([128, A16], I16, tag=f"{tg}a16")
        nc.sync.dma_start(aux16t[:], aux16_d[t, :, :])
        aux32t = sbb.tile([128, A32], I32, tag=f"{tg}a32")
        nc.sync.dma_start(aux32t[:], aux32_d[t, :, :])
        ldf = aux32t[:, 20:24].bitcast(F32)

        X4 = [
            sbb.tile([128, R, ROW], F32, tag=f"{tg}X{g}", name=f"{tg}X{g}_{t}")
            for g in range(GROUPS)
        ]
        for g in range(GROUPS):
            nc.gpsimd.dma_gather(
                out_ap=X4[g][:],
                in_ap=table[g * QROWS : (g + 1) * QROWS, :],
                idxs_ap=aux16t[:, 32 * g : 32 * (g + 1)],
                num_idxs=R * 128,
                num_idxs_reg=R * 128,
                elem_size=ROW,
                queue_num=g,
            )
        ARt = sbb.tile([128, GROUPS, ROW], F32, tag=f"{tg}AR")
        nc.gpsimd.dma_gather(
            out_ap=ARt[:],
            in_ap=loc_tab[:],
            idxs_ap=aux16t[:, 128:160],
            num_idxs=GROUPS * 128,
            num_idxs_reg=GROUPS * 128,
            elem_size=ROW,
            queue_num=(t % 4),
        )
        L = sbb.tile([128, LW], F32, tag=f"{tg}L")
        for g in range(GROUPS):
            nc.vector.tensor_tensor(
                L[:, g * R * H : (g + 1) * R * H].rearrange("p (r h) -> p r h", h=H),
                X4[g][:, :, CL : CL + H],
                ARt[:, g : g + 1, CR : CR + H].broadcast_to([128, R, H]),
                OP.add,
            )
        LM = sbb.tile([128, LW], F32, tag=f"{tg}LM")
        nc.vector.tensor_scalar_mul(LM[:], L[:], SLOPE)
        L2 = sbb.tile([128, LW], F32, tag=f"{tg}L2")
        nc.vector.tensor_tensor(L2[:], L[:], LM[:], OP.max)
        W = sbb.tile([128, LW], F32, tag=f"{tg}W")
        nc.scalar.activation(W[:], L2[:], AF.Exp)
        MSG = sbb.tile([128, BLOCKS * MW], F32, tag=f"{tg}MSG")
        for g in range(GROUPS):
            nc.vector.tensor_tensor(
                MSG[:, g * R * MW : (g + 1) * R * MW].rearrange(
                    "p (r rep h) -> p r rep h", r=R, h=H
                ),
                X4[g][:, :, 0:MW].rearrange("p r (rep h) -> p r rep h", h=H),
                W[:, g * R * H : (g + 1) * R * H]
                .rearrange("p (r h) -> p r h", h=H)[:, :, None, :]
                .broadcast_to([128, R, MW // H, H]),
                OP.mult,
            )
        P = pp.tile([128, MW], F32, tag="P")
        for g in range(GROUPS):
            O = sbb.tile([128, 128], F32, tag=f"{tg}O")
            nc.vector.tensor_scalar(
                O[:], iota_sb[:], ldf[:, g : g + 1], None, OP.is_equal
            )
            for r in range(R):
                b = R * g + r
                nc.tensor.matmul(
                    P[:], lhsT=O[:], rhs=MSG[:, b * MW : (b + 1) * MW],
                    start=(b == 0), stop=(b == BLOCKS - 1),
                )
        if dbg and t == 0 and layer == 1:
            nc.sync.dma_start(dbg["X0"][:], X4[0][:].rearrange("p r e -> p (r e)"))
            nc.sync.dma_start(dbg["AR0"][:], ARt[:].rearrange("p g e -> p (g e)"))
            nc.sync.dma_start(dbg["W0"][:], W[:])
            pc = sbb.tile([128, MW], F32, tag=f"{tg}pc")
            nc.vector.tensor_copy(pc[:], P[:])
            nc.sync.dma_start(dbg["P0"][:], pc[:])
        # epilogue
        D = sbb.tile([128, H], F32, tag=f"{tg}D")
        nc.vector.tensor_scalar(D[:], P[:, OUT:MW], 1e-16, None, OP.add)
        Rc = sbb.tile([128, H], F32, tag=f"{tg}R")
        nc.vector.reciprocal(Rc[:], D[:])
        icol = 160 if layer == 1 else 168
        xrw = OUT if layer == 1 else ROW
        XRt = sbb.tile([128, 1, xrw], F32, tag=f"{tg}XR")
        nc.gpsimd.dma_gather(
            out_ap=XRt[:],
            in_ap=xr_tab[:],
            idxs_ap=aux16t[:, icol : icol + 8],
            num_idxs=128, num_idxs_reg=128, elem_size=xrw,
            queue_num=((t + 2) % 4),
        )
        AG = sbb.tile([128, OUT], F32, tag=f"{tg}AG")
        if layer == 1:
            nc.vector.tensor_tensor(
                AG[:].rearrange("p (c h) -> p c h", h=H),
                P[:, 0:OUT].rearrange("p (c h) -> p c h", h=H),
                Rc[:, None, :].broadcast_to([128, HID, H]),
                OP.mult,
            )
        else:
            nc.vector.tensor_tensor(
                AG[:], P[:, 0:OUT], Rc[:].broadcast_to([128, OUT]), OP.mult
            )
        O1 = sbb.tile([128, OUT], F32, tag=f"{tg}O1")
        nc.vector.tensor_tensor(O1[:], AG[:], XRt[:, 0, 0:OUT], OP.add)
        Hh = sbb.tile([128, OUT], F32, tag=f"{tg}Hh")
        if layer == 1:
            Hm = sbb.tile([128, OUT], F32, tag=f"{tg}Hm")
            nc.vector.tensor_scalar_max(Hm[:], O1[:], 0.0)
            M0 = sbb.tile([128, OUT], F32, tag=f"{tg}M0")
            nc.vector.tensor_scalar_min(M0[:], O1[:], 0.0)
            E1 = sbb.tile([128, OUT], F32, tag=f"{tg}E1")
            nc.scalar.activation(E1[:], M0[:], AF.Exp)
            S1 = sbb.tile([128, OUT], F32, tag=f"{tg}S1")
            nc.vector.tensor_tensor(S1[:], Hm[:], E1[:], OP.add)
            nc.vector.tensor_scalar_add(Hh[:], S1[:], -1.0)
        else:
            nc.vector.tensor_tensor(Hh[:], O1[:], bias_sb[:], OP.add)
        nc.gpsimd.dma_scatter_add(
            out_ap=out_tab[:],
            in_ap=Hh[:, None, :],
            idxs_ap=aux16t[:, 160:168],
            num_idxs=128,
            num_idxs_reg=128,
            elem_size=OUT,
            queue_num=((t + 1) % 4),
        )


# --------------------------------------------------------------------------
# entry point
# --------------------------------------------------------------------------

def _run(inputs):
    x = np.asarray(inputs["x"], np.float32)
    edge_index = np.asarray(inputs["edge_index"], np.int32)
    W1 = np.asarray(inputs["W1"], np.float32)
    att_l1 = np.asarray(inputs["att_l1"], np.float32)
    att_r1 = np.asarray(inputs["att_r1"], np.float32)
    res_W1 = np.asarray(inputs["res_W1"], np.float32)
    b1 = np.asarray(inputs["b1"], np.float32).reshape(-1)
    W2 = np.asarray(inputs["W2"], np.float32)
    att_l2 = np.asarray(inputs["att_l2"], np.float32)
    att_r2 = np.asarray(inputs["att_r2"], np.float32)
    b2 = np.asarray(inputs["b2"], np.float32).reshape(-1)

    a16, a32, T = _prepare(edge_index)

    perm = np.arange(OUT).reshape(HEADS, HID).T.reshape(-1)
    W1p = np.ascontiguousarray(W1[:, perm])
    rw1p = np.ascontiguousarray(res_W1[:, perm])
    atl1_cm = att_l1.reshape(HEADS, HID).T.reshape(1, OUT)
    atr1_cm = att_r1.reshape(HEADS, HID).T.reshape(1, OUT)
    b1p = b1[perm].reshape(1, OUT)
    W2p = np.ascontiguousarray(W2[perm, :])

    iota = np.broadcast_to(np.arange(128, dtype=np.float32), (128, 128)).copy()
    ident = np.eye(128, dtype=np.float32)

    if T not in _CACHE:
        _CACHE[T] = _build(T)
    nc = _CACHE[T]

    in_maps = []
    for c in range(NC):
        xTc = np.zeros((128, HBLK), np.float32)
        xTc[:, :NLOC] = x[c * NLOC : (c + 1) * NLOC].T
        in_maps.append(
            {
                "xT": xTc,
                "aux16": a16[c],
                "aux32": a32[c],
                "w1": W1p,
                "rw1": rw1p,
                "atl1": np.tile(atl1_cm, (128, 1)),
                "atr1": np.tile(atr1_cm, (128, 1)),
                "b1": np.tile(b1p, (128, 1)),
                "w2": W2p,
                "atl2": np.tile(att_l2.reshape(1, OUT), (128, 1)),
                "atr2": np.tile(att_r2.reshape(1, OUT), (128, 1)),
                "b2": np.tile(b2.reshape(1, OUT), (128, 1)),
                "iota": iota,
                "ident": ident,
            }
        )
    return nc, in_maps


def kernel(**inputs):
    nc, in_maps = _run(inputs)
    res = bass_utils.run_bass_kernel_spmd(
        nc, in_maps, core_ids=list(range(NC)), trace=False
    )
    return np.concatenate(
        [res.results[c]["out"][:NLOC] for c in range(NC)], axis=0
    )
